# revision 29
# baseline (speedup 1.0000x reference)
"""Multi-headed attention kernel for Trainium2, SPMD across 8 NeuronCores.

Problem: B=4, S=2048, D_MODEL=1024, HEADS=16, D_HEAD=64 (fp32).

Sharding (per hint): batch across cores (4 batches x 2 cores each); within a
batch pair, heads are split 8+8 (tensor parallel). Each core computes, for its
(batch b, head half hh):
    Q^T = (Wq_s)^T X_q^T + bq   [512, 2048]   (hd-major layout, f32)
    K^T = (Wk_s)^T X_k^T + bk   [512, 2048]
    V'' = X_v Wv_s + bv         [2048, 8*65]  (bf16; per-head 65th col = 8.0)
    per head h, sq-half:  L^T = K_h Q_h^T  (f32r matmuls, PSUM f32)
                          P = exp(L^T + maskbias) (bf16; maskbias=-1e8 added in PSUM via identity matmul)
                          AV^T[65,sq] += V''_h^T P   (bf16 matmuls)
                          attn^T = AV^T[0:64] * (1 / AV^T[64])   (row 64 = 8*rowsum)
    out^T = Wo_s^T attn^T       [1024, 2048]  (partial over heads)
Host: out[b] = (outT_core0 + outT_core1).T + bo.

The mask bias (-1e8) is accumulated into the logits PSUM by an identity
matmul; exp then underflows to exactly 0, identical to the reference's where().
No row-max subtraction is needed: logits are O(+-50), exp stays finite in f32.
"""
import numpy as np
import ml_dtypes
from contextlib import ExitStack

import concourse.bass as bass
import concourse.tile as tile
from concourse import bacc, mybir
from concourse.bass_utils import run_bass_kernel_spmd

F32 = mybir.dt.float32
F32R = mybir.dt.float32r
BF16 = mybir.dt.bfloat16

B, S, D, H, DH = 4, 2048, 1024, 16, 64
HPC = 8           # heads per core
HD = HPC * DH     # 512 head-dims per core
NCORES = 8
ET = D // 128     # 8 e-tiles (d_model contraction tiles)
ST = S // 128     # 16 s-tiles
ADD = mybir.AluOpType.add

# stash of last run results for test harness introspection
last_results = None


def _emit(ctx: ExitStack, tc: tile.TileContext, io: dict):
    nc = tc.nc
    xqT, xkT, xvT = io["xqT"], io["xkT"], io["xvT"]
    wq, wk, wv, wo = io["wq"], io["wk"], io["wv"], io["wo"]
    bqh, bkh, bvh = io["bqh"], io["bkh"], io["bvh"]
    maskT, outT = io["maskT"], io["outT"]

    const = ctx.enter_context(tc.tile_pool(name="const", bufs=1))
    bigA = ctx.enter_context(tc.tile_pool(name="bigA", bufs=1))

    # ---- constants ----
    bqh_sb = const.tile([128, 4], F32)
    nc.sync.dma_start(bqh_sb, bqh)
    bkh_sb = const.tile([128, 4], F32)
    nc.sync.dma_start(bkh_sb, bkh)
    bv_bc = const.tile([128, HD], F32)
    nc.gpsimd.dma_start(bv_bc, bvh.partition_broadcast(128))
    idn = const.tile([128, 128], BF16)
    nc.sync.dma_start(idn, io["idn"])
    nbias = const.tile([128, 1], F32)
    nc.vector.memset(nbias, -131072.0)
    wo_sb = const.tile([128, 4, D], BF16)
    nc.sync.dma_start(wo_sb, io["wo"].rearrange("(j p) f -> p j f", p=128))

    # ---- tensors spanning phases A..C ----
    qT = bigA.tile([128, 4, S], F32R)       # [p, hd-tile, sq]
    kT = bigA.tile([128, 4, S], F32R)
    vs = bigA.tile([128, ST, HPC * 65], BF16)  # [p, s-tile, h*65+d]; col 64 per head = 8.0

    # mask tiles: 12 of 16 sk-tiles prefetched during phase A (dripped one
    # per c-block so the DMA rings stay fed but never starve the x loads);
    # the last 4 load at the start of phase B.  Keeping the PE from idling
    # >3.4us at the A->B seam stops the HAM clock-gate from re-throttling.
    NMA = 12
    mskp = ctx.enter_context(tc.tile_pool(name="mskp", bufs=1))
    mskA = mskp.tile([128, NMA, S], BF16)
    _mi = [0]

    def drip_msk():
        if _mi[0] < NMA:
            t = _mi[0]
            nc.sync.dma_start(mskA[:, t, :], maskT[t * 128:(t + 1) * 128, :])
            _mi[0] += 1

    # ===== Phase A: projections (x streamed per 512-col c-block) =====
    with tc.tile_pool(name="xin", bufs=18) as xin, \
         tc.tile_pool(name="wpool", bufs=10) as wpool, \
         tc.tile_pool(name="pa", bufs=4, space="PSUM") as pa:
        # --- Q^T and K^T (orientation: out[hd, sq] = W^T @ X^T) ---
        for which, (xT, w_dram, bias_sb, out_sb) in enumerate(
            [(xqT, wq, bqh_sb, qT), (xkT, wk, bkh_sb, kT)]
        ):
            w_es = []
            for e in range(ET):
                w_e = wpool.tile([128, HD], F32R, tag="w",
                                 name=f"w{which}_{e}")
                nc.sync.dma_start(w_e, w_dram[e * 128:(e + 1) * 128, :])
                w_es.append(w_e)
            for c in range(4):
                xts_c = []
                for e in range(ET):
                    xt = xin.tile([128, 512], F32R, tag="x",
                                  name=f"x{which}_{e}_{c}")
                    nc.sync.dma_start(
                        xt, xT[e * 128:(e + 1) * 128, c * 512:(c + 1) * 512])
                    xts_c.append(xt)
                for i in range(4):
                    ps = pa.tile([128, 512], F32, name="ps_qk")
                    for e in range(ET):
                        nc.tensor.matmul(
                            ps,
                            w_es[e][:, i * 128:(i + 1) * 128],
                            xts_c[e],
                            start=(e == 0), stop=(e == ET - 1),
                        )
                    nc.vector.tensor_scalar(
                        out=out_sb[:, i, c * 512:(c + 1) * 512], in0=ps,
                        scalar1=bias_sb[:, i:i + 1], scalar2=None, op0=ADD,
                    )
                drip_msk()

        # --- V'' (orientation: out[s, hd] = X @ Wv) ---
        wv_es = []
        for e in range(ET):
            w_e = wpool.tile([128, HD], F32R, tag="w", name=f"wv_{e}")
            nc.sync.dma_start(w_e, wv[e * 128:(e + 1) * 128, :])
            wv_es.append(w_e)
        ones_view = vs.rearrange("p s (h dd) -> p s h dd", dd=65)[:, :, :, 64:65]
        nc.vector.memset(ones_view, 8.0)
        for c in range(4):
            xv_c = []
            for e in range(ET):
                xt = xin.tile([128, 512], F32R, tag="x", name=f"xv_{e}_{c}")
                nc.sync.dma_start(
                    xt, xvT[e * 128:(e + 1) * 128, c * 512:(c + 1) * 512])
                xv_c.append(xt)
            for s in range(c * 4, c * 4 + 4):
                ps = pa.tile([128, 512], F32, name="ps_v")
                for e in range(ET):
                    nc.tensor.matmul(
                        ps,
                        xv_c[e][:, (s % 4) * 128:(s % 4 + 1) * 128],
                        wv_es[e],
                        start=(e == 0), stop=(e == ET - 1),
                    )
                nc.vector.tensor_add(
                    out=vs[:, s, :].rearrange(
                        "p (h dd) -> p h dd", dd=65)[:, :, 0:64],
                    in0=ps.rearrange("p (h d) -> p h d", d=64),
                    in1=bv_bc.rearrange("p (h d) -> p h d", d=64),
                )
            drip_msk()

    with tc.tile_pool(name="attp", bufs=1) as attp:
        att = attp.tile([128, 4, S], BF16)   # attn^T [hd, sq]

        # ===== Phase B: attention (phase C folded into the g=3 blocks) =====
        with tc.tile_pool(name="mskp2", bufs=1) as mskp2, \
             tc.tile_pool(name="ppool", bufs=4) as ppool, \
             tc.tile_pool(name="rpool", bufs=3) as rpool, \
             tc.tile_pool(name="rdram", bufs=3, space="DRAM") as rdram, \
             tc.tile_pool(name="ostg", bufs=4) as ostg, \
             tc.tile_pool(name="pqk", bufs=2, space="PSUM") as pqk, \
             tc.tile_pool(name="pav", bufs=3, space="PSUM") as pav, \
             tc.tile_pool(name="po", bufs=1, space="PSUM") as po:
            mskB = mskp2.tile([128, ST - NMA, S], BF16)
            for t in range(NMA, ST):
                nc.sync.dma_start(mskB[:, t - NMA, :],
                                  maskT[t * 128:(t + 1) * 128, :])

            def msk_t(t):
                return mskA[:, t, :] if t < NMA else mskB[:, t - NMA, :]

            # Output projection, one (dm, c) group at a time so the g=3
            # t-loops can interleave them between attention iterations
            # without starving ACT.  att[:, :, c-cols] is final once
            # (g=3, Hq=c) has normalized.
            out_backlog = []

            def emit_out_group():
                dm, c = out_backlog.pop(0)
                ps = po.tile([128, 512], F32, name="ps_o")
                for j in range(4):
                    nc.tensor.matmul(
                        ps,
                        wo_sb[:, j, dm * 128:(dm + 1) * 128],
                        att[:, j, c * 512:(c + 1) * 512],
                        start=(j == 0), stop=(j == 3),
                    )
                stg = ostg.tile([128, 512], F32, name="stg")
                nc.vector.tensor_copy(stg, ps)
                nc.sync.dma_start(
                    outT[dm * 128:(dm + 1) * 128, c * 512:(c + 1) * 512],
                    stg)

            for g in range(4):            # head pairs; bases 0/64 overlap on PE
                for Hq in range(4):       # sq quarters (512 cols)
                    q0 = Hq * 512
                    avs = []
                    for hl in range(2):
                        av = pav.tile([65, 512], F32, tag="av", name=f"av{hl}")
                        avs.append(av)

                    def emit_av(p_, t_):
                        for hl in range(2):
                            h = 2 * g + hl
                            nc.tensor.matmul(
                                avs[hl],
                                vs[:, t_, h * 65:(h + 1) * 65],
                                p_[:, hl * 512:(hl + 1) * 512],
                                start=(t_ == 0), stop=(t_ == ST - 1),
                            )

                    pending = None
                    for t in range(ST):
                        # One [h0|h1] logits tile; the two K=64 matmuls sit on
                        # row strips 0/64 of the PE.  The mask is a 0/1
                        # multiply: either on DVE after exp, or on the PE as
                        # qk += 2^17*m01 followed by exp(qk - 2^17) (masked
                        # entries underflow to 0; unmasked logits only lose
                        # 2^-7 of precision).  The PE path doubles as filler
                        # that keeps the PE saturated so the HAM clock gate
                        # never re-throttles it to 1.2 GHz.
                        pe_mask = g < 3 and t % 16 in (2, 5, 8, 11, 14)
                        qk = pqk.tile([128, 1024], F32, name="qk")
                        for hl in range(2):
                            r0 = hl * 64
                            nc.tensor.matmul(
                                qk[:, hl * 512:(hl + 1) * 512],
                                kT[r0:r0 + 64, g, t * 128:(t + 1) * 128],
                                qT[r0:r0 + 64, g, q0:q0 + 512],
                                start=True, stop=not pe_mask,
                            )
                        p = ppool.tile([128, 1024], BF16, name="p")
                        if pe_mask:
                            for hl in range(2):
                                nc.tensor.matmul(
                                    qk[:, hl * 512:(hl + 1) * 512],
                                    idn,
                                    msk_t(t)[:, q0:q0 + 512],
                                    start=False, stop=True,
                                )
                            nc.scalar.activation(
                                p, qk, mybir.ActivationFunctionType.Exp,
                                bias=nbias)
                        else:
                            nc.scalar.activation(
                                p, qk, mybir.ActivationFunctionType.Exp)
                            for hl in range(2):
                                nc.vector.tensor_mul(
                                    out=p[:, hl * 512:(hl + 1) * 512],
                                    in0=p[:, hl * 512:(hl + 1) * 512],
                                    in1=msk_t(t)[:, q0:q0 + 512],
                                )
                        # AV runs one t-step behind: by the time the PE pops
                        # these matmuls it has already streamed QK(t), so
                        # exp(t-1)+mask(t-1) are long done and the PE never
                        # blocks in-order on ACT/DVE.
                        if pending is not None:
                            emit_av(*pending)
                        pending = (p, t)
                        if t % 2 == 1 and out_backlog:
                            emit_out_group()
                    emit_av(*pending)
                    for hl in range(2):
                        r0 = hl * 64
                        av = avs[hl]
                        # Evacuate av quickly (frees the PSUM bank) then
                        # normalize.  Row 64 of av = 8*rowsum; reciprocal runs
                        # on a [64, 8] reshape (via DRAM bounce) since the
                        # iterative divide is ~8 cyc/elem per lane.
                        avc = rpool.tile([65, 512], F32, tag="avc", name="avc")
                        nc.vector.tensor_copy(avc, av)
                        rd = rdram.tile([1, 512], F32, tag="rd", name="rd")
                        nc.sync.dma_start(rd, avc[64:65, :])
                        rsq = rpool.tile([64, 8], F32, tag="rsq", name="rsq")
                        nc.sync.dma_start(
                            rsq, rd.rearrange("one (p j) -> (one p) j", j=8))
                        rsr = rpool.tile([64, 8], F32, tag="rsr", name="rsr")
                        nc.vector.reciprocal(rsr, rsq)
                        rd2 = rdram.tile([1, 512], F32, tag="rd2", name="rd2")
                        nc.sync.dma_start(
                            rd2.rearrange("one (p j) -> (one p) j", j=8), rsr)
                        rbc = rpool.tile([64, 512], F32, tag="rbc", name="rbc")
                        nc.gpsimd.dma_start(
                            rbc, rd2.partition_broadcast(64))
                        nc.vector.tensor_mul(
                            att[r0:r0 + 64, g, q0:q0 + 512], avc[0:64, :], rbc)
                    if g == 3:
                        out_backlog.extend((dm, Hq) for dm in range(ET))
            while out_backlog:
                emit_out_group()


def build_nc():
    nc = bacc.Bacc("TRN2", target_bir_lowering=False, debug=False,
                   num_devices=NCORES)
    io = {}
    for name, shape, dt_, kind in [
        ("xqT", [D, S], F32R, "ExternalInput"),
        ("xkT", [D, S], F32R, "ExternalInput"),
        ("xvT", [D, S], F32R, "ExternalInput"),
        ("wq", [D, HD], F32R, "ExternalInput"),
        ("wk", [D, HD], F32R, "ExternalInput"),
        ("wv", [D, HD], F32R, "ExternalInput"),
        ("wo", [HD, D], BF16, "ExternalInput"),
        ("bqh", [128, 4], F32, "ExternalInput"),
        ("bkh", [128, 4], F32, "ExternalInput"),
        ("bvh", [HD], F32, "ExternalInput"),
        ("maskT", [S, S], BF16, "ExternalInput"),
        ("idn", [128, 128], BF16, "ExternalInput"),
        ("outT", [D, S], F32, "ExternalOutput"),
    ]:
        io[name] = nc.dram_tensor(name, shape, dt_, kind=kind).ap()
    with tile.TileContext(nc) as tc:
        with ExitStack() as ctx:
            _emit(ctx, tc, io)
    nc.compile()
    return nc


def make_in_maps(query, key_, value, mask, Wq, bq, Wk, bk, Wv, bv, Wo, bo):
    in_maps = []
    for c in range(NCORES):
        b, hh = c // 2, c % 2
        h0 = hh * HPC
        mbT = np.ascontiguousarray(
            np.where(mask[b], np.float32(0.0), np.float32(1.0)).T
        ).astype(ml_dtypes.bfloat16)
        in_maps.append({
            "xqT": np.ascontiguousarray(query[b].T),
            "xkT": np.ascontiguousarray(key_[b].T),
            "xvT": np.ascontiguousarray(value[b].T),
            "wq": np.ascontiguousarray(Wq[:, h0:h0 + HPC, :].reshape(D, HD)),
            "wk": np.ascontiguousarray(Wk[:, h0:h0 + HPC, :].reshape(D, HD)),
            "wv": np.ascontiguousarray(Wv[:, h0:h0 + HPC, :].reshape(D, HD)),
            "wo": np.ascontiguousarray(
                Wo[h0:h0 + HPC].reshape(HD, D)).astype(ml_dtypes.bfloat16),
            "bqh": np.ascontiguousarray(
                bq[h0:h0 + HPC].reshape(4, 128).T),
            "bkh": np.ascontiguousarray(
                bk[h0:h0 + HPC].reshape(4, 128).T),
            "bvh": np.ascontiguousarray(bv[h0:h0 + HPC].reshape(HD)),
            "maskT": mbT,
            "idn": (np.eye(128, dtype=np.float32) * 131072.0
                    ).astype(ml_dtypes.bfloat16),
        })
    return in_maps


_nc_cache = None


def kernel(query, key_, value, mask, Wq, bq, Wk, bk, Wv, bv, Wo, bo):
    global last_results, _nc_cache
    query = np.asarray(query, dtype=np.float32)
    key_ = np.asarray(key_, dtype=np.float32)
    value = np.asarray(value, dtype=np.float32)
    mask = np.asarray(mask, dtype=bool)
    Wq, bq = np.asarray(Wq, np.float32), np.asarray(bq, np.float32)
    Wk, bk = np.asarray(Wk, np.float32), np.asarray(bk, np.float32)
    Wv, bv = np.asarray(Wv, np.float32), np.asarray(bv, np.float32)
    Wo, bo = np.asarray(Wo, np.float32), np.asarray(bo, np.float32)

    if _nc_cache is None:
        _nc_cache = build_nc()
    in_maps = make_in_maps(query, key_, value, mask, Wq, bq, Wk, bk,
                           Wv, bv, Wo, bo)
    res = run_bass_kernel_spmd(_nc_cache, in_maps, core_ids=list(range(NCORES)))
    last_results = res
    out = np.empty((B, S, D), dtype=np.float32)
    for b in range(B):
        acc = res.results[2 * b]["outT"].astype(np.float32) + \
            res.results[2 * b + 1]["outT"].astype(np.float32)
        out[b] = acc.T + bo[None, :]
    return out



# revision 30
# speedup vs baseline: 1.1434x; 1.1434x over previous
"""Multi-headed attention kernel for Trainium2, SPMD across 8 NeuronCores.

Problem: B=4, S=2048, D_MODEL=1024, HEADS=16, D_HEAD=64 (fp32).

Sharding (per hint): batch across cores (4 batches x 2 cores each); within a
batch pair, heads are split 8+8 (tensor parallel). Each core computes, for its
(batch b, head half hh):
    Q^T = (Wq_s)^T X_q^T + bq   [512, 2048]   (hd-major layout, f32r)
    K^T = (Wk_s)^T X_k^T + bk   [512, 2048]
    V'' = X_v Wv_s + bv         [2048, 8*65]  (bf16; per-head 65th col = 8.0)
    per head h, sq-quarter: L^T = K_h Q_h^T   (f32r matmuls, PSUM f32)
                            P = exp(L^T) * mask01  (bf16)
                            AV^T[65,sq] += V''_h^T P   (bf16 matmuls)
                            attn^T = AV^T[0:64] / AV^T[64]  (row 64 = 8*rowsum)
    out^T = Wo_s^T attn^T       [1024, 2048]  (partial over heads)
Host: out[b] = (outT_core0 + outT_core1).T + bo.

Masking: mask01 is 0/1 in bf16.  Most t-steps apply it as a DVE multiply
after exp; a fraction apply it on the PE (qk += 2^17*mask01 via a scaled
identity matmul, then exp(qk - 2^17): masked entries underflow to exactly 0,
unmasked logits only lose 2^-7 of mantissa).  The PE path doubles as filler
that keeps the PE saturated so the HAM clock gate never re-throttles it to
1.2 GHz.  Phase B runs a flat software pipeline over (g, Hq, t): the AV
matmuls trail the QK/exp front by 2 steps (so exp+mask are always done when
the PE pops them), crossing block boundaries without ever idling the PE; the
per-block normalize + the g=3 output-projection groups are interleaved into
the following block's t-steps.
"""
import numpy as np
import ml_dtypes
from contextlib import ExitStack

import concourse.bass as bass
import concourse.tile as tile
from concourse import bacc, mybir
from concourse.bass_utils import run_bass_kernel_spmd

F32 = mybir.dt.float32
F32R = mybir.dt.float32r
BF16 = mybir.dt.bfloat16

B, S, D, H, DH = 4, 2048, 1024, 16, 64
HPC = 8           # heads per core
HD = HPC * DH     # 512 head-dims per core
NCORES = 8
ET = D // 128     # 8 e-tiles (d_model contraction tiles)
ST = S // 128     # 16 s-tiles
ADD = mybir.AluOpType.add
PE_MASK_TS = (2, 5, 8, 11, 14)   # t-steps whose mask rides the PE (filler)

# stash of last run results for test harness introspection
last_results = None


def _emit(ctx: ExitStack, tc: tile.TileContext, io: dict):
    nc = tc.nc
    xqT, xkT, xvT = io["xqT"], io["xkT"], io["xvT"]
    wq, wk, wv = io["wq"], io["wk"], io["wv"]
    bqh, bkh, bvh = io["bqh"], io["bkh"], io["bvh"]
    maskT, outT = io["maskT"], io["outT"]

    const = ctx.enter_context(tc.tile_pool(name="const", bufs=1))
    bigA = ctx.enter_context(tc.tile_pool(name="bigA", bufs=1))

    # ---- constants ----
    bqh_sb = const.tile([128, 4], F32)
    nc.sync.dma_start(bqh_sb, bqh)
    bkh_sb = const.tile([128, 4], F32)
    nc.sync.dma_start(bkh_sb, bkh)
    bv_bc = const.tile([128, HD], F32)
    nc.gpsimd.dma_start(bv_bc, bvh.partition_broadcast(128))
    idn = const.tile([128, 128], BF16)
    nc.sync.dma_start(idn, io["idn"])
    nbias = const.tile([128, 1], F32)
    nc.vector.memset(nbias, -131072.0)
    wo_sb = const.tile([128, 4, D], BF16)
    nc.sync.dma_start(wo_sb, io["wo"].rearrange("(j p) f -> p j f", p=128))

    # ---- tensors spanning phases A..C ----
    qT = bigA.tile([128, 4, S], F32R)       # [p, hd-tile, sq]
    kT = bigA.tile([128, 4, S], F32R)
    vs = bigA.tile([128, ST, HPC * 65], BF16)  # [p, s-tile, h*65+d]; col 64 per head = 8.0

    # ===== Phase A: projections =====
    with tc.tile_pool(name="xin", bufs=36) as xin, \
         tc.tile_pool(name="wpool", bufs=2) as wpool, \
         tc.tile_pool(name="pa", bufs=4, space="PSUM") as pa:
        # --- Q^T and K^T (orientation: out[hd, sq] = W^T @ X^T) ---
        for which, (xT, w_dram, bias_sb, out_sb) in enumerate(
            [(xqT, wq, bqh_sb, qT), (xkT, wk, bkh_sb, kT)]
        ):
            w_sb = wpool.tile([128, ET, HD], F32R, tag="w", name=f"w{which}")
            for e in range(ET):
                nc.sync.dma_start(w_sb[:, e, :],
                                  w_dram[e * 128:(e + 1) * 128, :])
            # column-chunked loads: the first matmul group only waits on the
            # first 8 chunks (2 MB), not the whole 8 MB input
            xts = {}
            for c in range(4):
                for e in range(ET):
                    xt = xin.tile([128, 512], F32R, tag="x",
                                  name=f"x{which}_{e}_{c}")
                    nc.sync.dma_start(
                        xt, xT[e * 128:(e + 1) * 128, c * 512:(c + 1) * 512])
                    xts[(e, c)] = xt
            for c in range(4):
                for i in range(4):
                    ps = pa.tile([128, 512], F32, name="ps_qk")
                    for e in range(ET):
                        nc.tensor.matmul(
                            ps,
                            w_sb[:, e, i * 128:(i + 1) * 128],
                            xts[(e, c)],
                            start=(e == 0), stop=(e == ET - 1),
                        )
                    nc.vector.tensor_scalar(
                        out=out_sb[:, i, c * 512:(c + 1) * 512], in0=ps,
                        scalar1=bias_sb[:, i:i + 1], scalar2=None, op0=ADD,
                    )

        # --- V'' (orientation: out[s, hd] = X @ Wv) ---
        wv_sb = wpool.tile([128, ET, HD], F32R, tag="w")
        for e in range(ET):
            nc.sync.dma_start(wv_sb[:, e, :], wv[e * 128:(e + 1) * 128, :])
        xvs = {}
        for c in range(4):
            for e in range(ET):
                xt = xin.tile([128, 512], F32R, tag="x", name=f"xv_{e}_{c}")
                nc.sync.dma_start(
                    xt, xvT[e * 128:(e + 1) * 128, c * 512:(c + 1) * 512])
                xvs[(e, c)] = xt
        ones_view = vs.rearrange("p s (h dd) -> p s h dd", dd=65)[:, :, :, 64:65]
        nc.vector.memset(ones_view, 8.0)
        for s in range(ST):
            ps = pa.tile([128, 512], F32, name="ps_v")
            for e in range(ET):
                nc.tensor.matmul(
                    ps,
                    xvs[(e, s // 4)][:, (s % 4) * 128:(s % 4 + 1) * 128],
                    wv_sb[:, e, :],
                    start=(e == 0), stop=(e == ET - 1),
                )
            nc.vector.tensor_add(
                out=vs[:, s, :].rearrange("p (h dd) -> p h dd", dd=65)[:, :, 0:64],
                in0=ps.rearrange("p (h d) -> p h d", d=64),
                in1=bv_bc.rearrange("p (h d) -> p h d", d=64),
            )

    with tc.tile_pool(name="attp", bufs=1) as attp:
        att = attp.tile([128, 4, S], BF16)   # attn^T [hd, sq]

        # ===== Phase B: flat attention pipeline (C folded into g=3) =====
        with tc.tile_pool(name="mskp", bufs=1) as mskp, \
             tc.tile_pool(name="ppool", bufs=4) as ppool, \
             tc.tile_pool(name="rpool", bufs=3) as rpool, \
             tc.tile_pool(name="rdram", bufs=3, space="DRAM") as rdram, \
             tc.tile_pool(name="ostg", bufs=4) as ostg, \
             tc.tile_pool(name="pqk", bufs=2, space="PSUM") as pqk, \
             tc.tile_pool(name="pav", bufs=3, space="PSUM") as pav, \
             tc.tile_pool(name="po", bufs=1, space="PSUM") as po:
            msk = mskp.tile([128, ST, S], BF16)   # mask01^T [sk, sq]
            for t in range(ST):
                nc.sync.dma_start(msk[:, t, :], maskT[t * 128:(t + 1) * 128, :])

            out_backlog = []

            def emit_out_group(tail=False):
                # one (dm, c) group of the output projection; at the very
                # tail the free qk PSUM slots double the pipelining depth
                dm, c = out_backlog.pop(0)
                if tail and dm % 2 == 0:
                    psq = pqk.tile([128, 1024], F32, name="qk")
                    ps = psq[:, 0:512]
                else:
                    ps = po.tile([128, 512], F32, name="ps_o")
                for j in range(4):
                    nc.tensor.matmul(
                        ps,
                        wo_sb[:, j, dm * 128:(dm + 1) * 128],
                        att[:, j, c * 512:(c + 1) * 512],
                        start=(j == 0), stop=(j == 3),
                    )
                stg = ostg.tile([128, 512], F32, name="stg")
                nc.vector.tensor_copy(stg, ps)
                nc.sync.dma_start(
                    outT[dm * 128:(dm + 1) * 128, c * 512:(c + 1) * 512],
                    stg)

            def do_normalize(avs_, g_, q0_):
                # attn = av[0:64] * (1 / av[64]); the reciprocal of the
                # [1, 512] rowsum row runs on a [64, 8] reshape via a DRAM
                # bounce (iterative divide is ~8 cyc/elem/lane)
                for hl in range(2):
                    r0 = hl * 64
                    av = avs_[hl]
                    avc = rpool.tile([65, 512], F32, tag="avc", name="avc")
                    nc.vector.tensor_copy(avc, av)
                    rd = rdram.tile([1, 512], F32, tag="rd", name="rd")
                    nc.sync.dma_start(rd, avc[64:65, :])
                    rsq = rpool.tile([64, 8], F32, tag="rsq", name="rsq")
                    nc.sync.dma_start(
                        rsq, rd.rearrange("one (p j) -> (one p) j", j=8))
                    rsr = rpool.tile([64, 8], F32, tag="rsr", name="rsr")
                    nc.vector.reciprocal(rsr, rsq)
                    rd2 = rdram.tile([1, 512], F32, tag="rd2", name="rd2")
                    nc.sync.dma_start(
                        rd2.rearrange("one (p j) -> (one p) j", j=8), rsr)
                    rbc = rpool.tile([64, 512], F32, tag="rbc", name="rbc")
                    nc.gpsimd.dma_start(rbc, rd2.partition_broadcast(64))
                    nc.vector.tensor_mul(
                        att[r0:r0 + 64, g_, q0_:q0_ + 512], avc[0:64, :], rbc)

            pend = []   # (avs, g, Hq, p, t) AV work trailing the QK front

            def pop_av():
                avs_, g_, Hq_, p_, t_ = pend.pop(0)
                for hl in range(2):
                    h = 2 * g_ + hl
                    nc.tensor.matmul(
                        avs_[hl],
                        vs[:, t_, h * 65:(h + 1) * 65],
                        p_[:, hl * 512:(hl + 1) * 512],
                        start=(t_ == 0), stop=(t_ == ST - 1),
                    )
                if t_ == ST - 1:
                    do_normalize(avs_, g_, Hq_ * 512)
                    if g_ == 3:
                        out_backlog.extend((dm, Hq_) for dm in range(ET))

            for g in range(4):            # head pairs; bases 0/64 on the PE
                for Hq in range(4):       # sq quarters (512 cols)
                    q0 = Hq * 512
                    avs = []
                    for hl in range(2):
                        av = pav.tile([65, 512], F32, tag="av",
                                      name=f"av{hl}")
                        avs.append(av)
                    for t in range(ST):
                        pe_mask = g < 3 and t in PE_MASK_TS
                        qk = pqk.tile([128, 1024], F32, name="qk")
                        for hl in range(2):
                            r0 = hl * 64
                            nc.tensor.matmul(
                                qk[:, hl * 512:(hl + 1) * 512],
                                kT[r0:r0 + 64, g, t * 128:(t + 1) * 128],
                                qT[r0:r0 + 64, g, q0:q0 + 512],
                                start=True, stop=not pe_mask,
                            )
                        p = ppool.tile([128, 1024], BF16, name="p")
                        if pe_mask:
                            for hl in range(2):
                                nc.tensor.matmul(
                                    qk[:, hl * 512:(hl + 1) * 512],
                                    idn,
                                    msk[:, t, q0:q0 + 512],
                                    start=False, stop=True,
                                )
                            nc.scalar.activation(
                                p, qk, mybir.ActivationFunctionType.Exp,
                                bias=nbias)
                        else:
                            nc.scalar.activation(
                                p, qk, mybir.ActivationFunctionType.Exp)
                            for hl in range(2):
                                nc.vector.tensor_mul(
                                    out=p[:, hl * 512:(hl + 1) * 512],
                                    in0=p[:, hl * 512:(hl + 1) * 512],
                                    in1=msk[:, t, q0:q0 + 512],
                                )
                        pend.append((avs, g, Hq, p, t))
                        if len(pend) > 2:
                            pop_av()
                        if out_backlog and t % 2 == 1:
                            emit_out_group()
            while pend:
                pop_av()
            while out_backlog:
                emit_out_group(tail=True)


def build_nc():
    nc = bacc.Bacc("TRN2", target_bir_lowering=False, debug=False,
                   num_devices=NCORES)
    io = {}
    for name, shape, dt_, kind in [
        ("xqT", [D, S], F32R, "ExternalInput"),
        ("xkT", [D, S], F32R, "ExternalInput"),
        ("xvT", [D, S], F32R, "ExternalInput"),
        ("wq", [D, HD], F32R, "ExternalInput"),
        ("wk", [D, HD], F32R, "ExternalInput"),
        ("wv", [D, HD], F32R, "ExternalInput"),
        ("wo", [HD, D], BF16, "ExternalInput"),
        ("bqh", [128, 4], F32, "ExternalInput"),
        ("bkh", [128, 4], F32, "ExternalInput"),
        ("bvh", [HD], F32, "ExternalInput"),
        ("maskT", [S, S], BF16, "ExternalInput"),
        ("idn", [128, 128], BF16, "ExternalInput"),
        ("outT", [D, S], F32, "ExternalOutput"),
    ]:
        io[name] = nc.dram_tensor(name, shape, dt_, kind=kind).ap()
    with tile.TileContext(nc) as tc:
        with ExitStack() as ctx:
            _emit(ctx, tc, io)
    nc.compile()
    return nc


def make_in_maps(query, key_, value, mask, Wq, bq, Wk, bk, Wv, bv, Wo, bo):
    in_maps = []
    for c in range(NCORES):
        b, hh = c // 2, c % 2
        h0 = hh * HPC
        mbT = np.ascontiguousarray(
            np.where(mask[b], np.float32(0.0), np.float32(1.0)).T
        ).astype(ml_dtypes.bfloat16)
        in_maps.append({
            "xqT": np.ascontiguousarray(query[b].T),
            "xkT": np.ascontiguousarray(key_[b].T),
            "xvT": np.ascontiguousarray(value[b].T),
            "wq": np.ascontiguousarray(Wq[:, h0:h0 + HPC, :].reshape(D, HD)),
            "wk": np.ascontiguousarray(Wk[:, h0:h0 + HPC, :].reshape(D, HD)),
            "wv": np.ascontiguousarray(Wv[:, h0:h0 + HPC, :].reshape(D, HD)),
            "wo": np.ascontiguousarray(
                Wo[h0:h0 + HPC].reshape(HD, D)).astype(ml_dtypes.bfloat16),
            "bqh": np.ascontiguousarray(
                bq[h0:h0 + HPC].reshape(4, 128).T),
            "bkh": np.ascontiguousarray(
                bk[h0:h0 + HPC].reshape(4, 128).T),
            "bvh": np.ascontiguousarray(bv[h0:h0 + HPC].reshape(HD)),
            "maskT": mbT,
            "idn": (np.eye(128, dtype=np.float32) * 131072.0
                    ).astype(ml_dtypes.bfloat16),
        })
    return in_maps


_nc_cache = None


def kernel(query, key_, value, mask, Wq, bq, Wk, bk, Wv, bv, Wo, bo):
    global last_results, _nc_cache
    query = np.asarray(query, dtype=np.float32)
    key_ = np.asarray(key_, dtype=np.float32)
    value = np.asarray(value, dtype=np.float32)
    mask = np.asarray(mask, dtype=bool)
    Wq, bq = np.asarray(Wq, np.float32), np.asarray(bq, np.float32)
    Wk, bk = np.asarray(Wk, np.float32), np.asarray(bk, np.float32)
    Wv, bv = np.asarray(Wv, np.float32), np.asarray(bv, np.float32)
    Wo, bo = np.asarray(Wo, np.float32), np.asarray(bo, np.float32)

    if _nc_cache is None:
        _nc_cache = build_nc()
    in_maps = make_in_maps(query, key_, value, mask, Wq, bq, Wk, bk,
                           Wv, bv, Wo, bo)
    res = run_bass_kernel_spmd(_nc_cache, in_maps, core_ids=list(range(NCORES)))
    last_results = res
    out = np.empty((B, S, D), dtype=np.float32)
    for b in range(B):
        acc = res.results[2 * b]["outT"].astype(np.float32) + \
            res.results[2 * b + 1]["outT"].astype(np.float32)
        out[b] = acc.T + bo[None, :]
    return out
